# revision 6
# baseline (speedup 1.0000x reference)
"""MoE (2-expert SwiGLU) Trainium2 kernel, 8-core SPMD.

Strategy: since the MLPs have no biases and silu(0) = 0, MLP(0) = 0, so each
token only needs the expert it is routed to.  The host gathers tokens by
expert (MoE dispatch), cores 0-3 process expert-0 tokens and cores 4-7
expert-1 tokens (~1/8 of total tokens per core), each core running a dense
SwiGLU MLP with its expert's weights.  The host scatters per-core outputs
back into the full [B, S, D] output.  This halves FLOPs vs. the reference's
dense-masked formulation and needs no collectives.

Device dataflow (per core, transposed so no on-chip transposes are needed):
  yT = Wd^T @ (silu(Wg^T @ xT) * (Wu^T @ xT))
Weights are the stationary matmul operand, token-columns the moving operand.
All matmuls are bf16 with fp32 PSUM accumulation.  The FF intermediate `h`
for all of a core's tokens stays resident in SBUF, so each weight byte is
DMA'd exactly once per core.
"""

import sys

for _p in ("/opt/trn_rl_repo", "/root/.axon_site/_ro/trn_rl_repo"):
    if _p not in sys.path:
        sys.path.append(_p)

import numpy as np
import ml_dtypes

BF16 = ml_dtypes.bfloat16

D_MODEL = 1024
D_FF = 4096
P = 128
KD = D_MODEL // P  # 8   k-tiles over d_model
MF = D_FF // P     # 32  tiles over d_ff
N_CORES = 8
CPE = 4            # cores per expert
NT = 3             # token tiles per core

_program_cache: dict[int, object] = {}


def _build_program(TS: int):
    """Bass program for one core: x [D,C] -> y [D,C], C = NT*TS tokens."""
    import concourse.tile as tile
    from concourse import mybir, bacc

    C = NT * TS
    f32 = mybir.dt.float32
    b16 = mybir.dt.bfloat16

    nc = bacc.Bacc()
    xT = nc.declare_dram_parameter("xT", [P, KD, C], b16, isOutput=False)
    # w1[mf, p, gu, kd, c] = (wg if gu==0 else wu)[kd*128 + p, mf*128 + c]
    w1 = nc.declare_dram_parameter("w1", [MF, P, 2, KD, P], b16, isOutput=False)
    # wdp[md, p, kf, c] = wd[kf*128 + p, md*128 + c]
    wdp = nc.declare_dram_parameter("wd", [KD, P, MF, P], b16, isOutput=False)
    yT = nc.declare_dram_parameter("yT", [KD, P, C], f32, isOutput=True)

    with tile.TileContext(nc) as tc:
        with (
            tc.tile_pool(name="xp", bufs=1) as xp,
            tc.tile_pool(name="hp", bufs=1) as hp,
            tc.tile_pool(name="w1p", bufs=3) as w1p,
            tc.tile_pool(name="wdpool", bufs=2) as wdpool,
            tc.tile_pool(name="silp", bufs=4) as silp,
            tc.tile_pool(name="yp", bufs=2) as yp,
        ):
            x_sb = xp.tile([P, KD, C], b16)
            nc.sync.dma_start(x_sb[:], xT[:])
            h_sb = hp.tile([P, MF, C], b16)

            # Stage 1: h = silu(Wg^T x) * (Wu^T x), laid out [ff-part, C]
            with tc.tile_pool(name="ps1", bufs=NT, space="PSUM") as ps1:
                for mf in range(MF):
                    wt = w1p.tile([P, 2, KD, P], b16)
                    nc.gpsimd.dma_start(wt[:], w1[mf])
                    psg = [ps1.tile([P, 512], f32, tag="psg", name=f"psg_{mf}_{t}")
                           for t in range(NT)]
                    psu = [ps1.tile([P, 512], f32, tag="psu", name=f"psu_{mf}_{t}")
                           for t in range(NT)]
                    for kd in range(KD):
                        for gu in range(2):
                            ps = psg if gu == 0 else psu
                            for t in range(NT):
                                nc.tensor.matmul(
                                    ps[t][:, :TS],
                                    wt[:, gu, kd],
                                    x_sb[:, kd, t * TS:(t + 1) * TS],
                                    start=(kd == 0),
                                    stop=(kd == KD - 1),
                                )
                    for t in range(NT):
                        sil = silp.tile([P, TS], f32)
                        nc.scalar.activation(
                            sil[:], psg[t][:, :TS],
                            mybir.ActivationFunctionType.Silu,
                        )
                        nc.vector.tensor_mul(
                            h_sb[:, mf, t * TS:(t + 1) * TS],
                            sil[:], psu[t][:, :TS],
                        )

            # Stage 2: y = Wd^T h, laid out [d-part, C]
            with tc.tile_pool(name="ps2", bufs=NT, space="PSUM") as ps2:
                for md in range(KD):
                    wdt = wdpool.tile([P, MF, P], b16)
                    nc.gpsimd.dma_start(wdt[:], wdp[md])
                    y_sb = yp.tile([P, C], f32)
                    psy = [ps2.tile([P, 512], f32, tag="psy", name=f"psy_{md}_{t}")
                           for t in range(NT)]
                    for kf in range(MF):
                        for t in range(NT):
                            nc.tensor.matmul(
                                psy[t][:, :TS],
                                wdt[:, kf],
                                h_sb[:, kf, t * TS:(t + 1) * TS],
                                start=(kf == 0),
                                stop=(kf == MF - 1),
                            )
                    for t in range(NT):
                        nc.vector.tensor_copy(
                            y_sb[:, t * TS:(t + 1) * TS], psy[t][:, :TS]
                        )
                    nc.gpsimd.dma_start(yT[md], y_sb[:])

    nc.compile()
    return nc


def _pack_w1(wg: np.ndarray, wu: np.ndarray) -> np.ndarray:
    """[D, F] x2 -> [MF, P, 2, KD, P] bf16, matching the kernel's layout."""
    # w1[mf, p, gu, kd, c] = w_gu[kd*128 + p, mf*128 + c]
    stack = np.stack([wg, wu], axis=0)            # [2, D, F]
    r = stack.reshape(2, KD, P, MF, P)            # [gu, kd, p, mf, c]
    return np.ascontiguousarray(r.transpose(3, 2, 0, 1, 4)).astype(BF16)


def _pack_wd(wd: np.ndarray) -> np.ndarray:
    """[F, D] -> [KD, P, MF, P] bf16. wdp[md, p, kf, c] = wd[kf*128+p, md*128+c]"""
    r = wd.reshape(MF, P, KD, P)                  # [kf, p, md, c]
    return np.ascontiguousarray(r.transpose(2, 1, 0, 3)).astype(BF16)


def _run_device(in_maps, TS):
    from concourse.bass_utils import run_bass_kernel_spmd

    key = TS
    if key not in _program_cache:
        _program_cache[key] = _build_program(TS)
    nc = _program_cache[key]
    res = run_bass_kernel_spmd(nc, in_maps, core_ids=list(range(N_CORES)))
    return [r["yT"] for r in res.results]


def kernel(hidden_states, routing_mask, wg0, wu0, wd0, wg1, wu1, wd1,
           _run=None):
    hidden_states = np.asarray(hidden_states, dtype=np.float32)
    routing_mask = np.asarray(routing_mask)
    B, S, D = hidden_states.shape
    NTOK = B * S
    x = hidden_states.reshape(NTOK, D)
    mask = routing_mask.reshape(NTOK)

    idx = [np.nonzero(mask == e)[0] for e in (0, 1)]
    maxpc = max(
        (len(idx[0]) + CPE - 1) // CPE,
        (len(idx[1]) + CPE - 1) // CPE,
        1,
    )
    # token tile size: multiple of 32, NT tiles per core, <= 512 per tile
    TS = 32 * ((maxpc + NT * 32 - 1) // (NT * 32))
    assert TS <= 512, f"too many tokens per core ({maxpc})"
    C = NT * TS

    w1_packed = [_pack_w1(np.asarray(wg0), np.asarray(wu0)),
                 _pack_w1(np.asarray(wg1), np.asarray(wu1))]
    wd_packed = [_pack_wd(np.asarray(wd0)), _pack_wd(np.asarray(wd1))]

    in_maps = []
    chunks = []  # (expert, token_indices) per core
    for core in range(N_CORES):
        e = core // CPE
        slot = core % CPE
        ids = idx[e]
        # split ids into CPE nearly-equal chunks
        bounds = [(len(ids) * i) // CPE for i in range(CPE + 1)]
        ids_c = ids[bounds[slot]:bounds[slot + 1]]
        chunks.append((e, ids_c))

        xc = np.zeros((C, D), dtype=np.float32)
        xc[: len(ids_c)] = x[ids_c]
        # xT[p, kd, c] = xc[c, kd*128 + p]
        xT = np.ascontiguousarray(
            xc.reshape(C, KD, P).transpose(2, 1, 0)
        ).astype(BF16)
        in_maps.append({
            "xT": xT,
            "w1": w1_packed[e],
            "wd": wd_packed[e],
        })

    run = _run if _run is not None else _run_device
    outs = run(in_maps, TS)

    y_full = np.zeros((NTOK, D), dtype=np.float32)
    for core in range(N_CORES):
        _, ids_c = chunks[core]
        if len(ids_c) == 0:
            continue
        yT = np.asarray(outs[core], dtype=np.float32).reshape(D, C)
        y_full[ids_c] = yT[:, : len(ids_c)].T
    return y_full.reshape(B, S, D)


# revision 8
# speedup vs baseline: 1.0268x; 1.0268x over previous
"""MoE (2-expert SwiGLU) Trainium2 kernel, 8-core SPMD.

Strategy: since the MLPs have no biases and silu(0) = 0, MLP(0) = 0, so each
token only needs the expert it is routed to.  The host gathers tokens by
expert (MoE dispatch), cores 0-3 process expert-0 tokens and cores 4-7
expert-1 tokens (~1/8 of total tokens per core), each core running a dense
SwiGLU MLP with its expert's weights.  The host scatters per-core outputs
back into the full [B, S, D] output.  This halves FLOPs vs. the reference's
dense-masked formulation and needs no collectives.

Device dataflow (per core, transposed so no on-chip transposes are needed):
  yT = Wd^T @ (silu(Wg^T @ xT) * (Wu^T @ xT))
Weights are the stationary matmul operand, token-columns the moving operand.
All matmuls are bf16 with fp32 PSUM accumulation.  The FF intermediate `h`
for all of a core's tokens stays resident in SBUF, so each weight byte is
DMA'd exactly once per core.
"""

import sys

for _p in ("/opt/trn_rl_repo", "/root/.axon_site/_ro/trn_rl_repo"):
    if _p not in sys.path:
        sys.path.append(_p)

import numpy as np
import ml_dtypes

BF16 = ml_dtypes.bfloat16

D_MODEL = 1024
D_FF = 4096
P = 128
KD = D_MODEL // P  # 8   k-tiles over d_model
MF = D_FF // P     # 32  tiles over d_ff
N_CORES = 8
CPE = 4            # cores per expert
NT = 3             # token tiles per core

_program_cache: dict[tuple, object] = {}


def _token_tiles(maxpc: int) -> tuple:
    """Split the per-core token capacity into NT near-equal tiles (each a
    multiple of 8 except possibly the last, each <= 512)."""
    C = 8 * ((maxpc + 7) // 8)
    C = max(C, 24)
    t = 8 * ((C + NT * 8 - 1) // (NT * 8))    # per-tile, rounded up to 8
    tiles = []
    left = C
    for _ in range(NT):
        s = min(t, left)
        tiles.append(s)
        left -= s
    assert left == 0 and all(0 < s <= 512 for s in tiles), (maxpc, tiles)
    return tuple(tiles)


def _build_program(tiles: tuple):
    """Bass program for one core: x [D,C] -> y [D,C], C = sum(tiles) tokens."""
    import concourse.tile as tile
    from concourse import mybir, bacc

    C = sum(tiles)
    offs = [sum(tiles[:i]) for i in range(len(tiles))]
    TSMAX = max(tiles)
    f32 = mybir.dt.float32
    b16 = mybir.dt.bfloat16

    nc = bacc.Bacc()
    xT = nc.declare_dram_parameter("xT", [P, KD, C], b16, isOutput=False)
    # w1[mf, p, gu, kd, c] = (wg if gu==0 else wu)[kd*128 + p, mf*128 + c]
    w1 = nc.declare_dram_parameter("w1", [MF, P, 2, KD, P], b16, isOutput=False)
    # wdp[md, p, kf, c] = wd[kf*128 + p, md*128 + c]
    wdp = nc.declare_dram_parameter("wd", [KD, P, MF, P], b16, isOutput=False)
    yT = nc.declare_dram_parameter("yT", [KD, P, C], b16, isOutput=True)

    with tile.TileContext(nc) as tc:
        with (
            tc.tile_pool(name="xp", bufs=1) as xp,
            tc.tile_pool(name="hp", bufs=1) as hp,
            tc.tile_pool(name="w1p", bufs=3) as w1p,
            tc.tile_pool(name="wdpool", bufs=2) as wdpool,
            tc.tile_pool(name="silp", bufs=4) as silp,
            tc.tile_pool(name="yp", bufs=2) as yp,
        ):
            x_sb = xp.tile([P, KD, C], b16)
            # split the x load per k-slice so the PE can start after the
            # first slice + first weight tile land
            for kd in range(KD):
                nc.sync.dma_start(x_sb[:, kd], xT[:, kd])
            h_sb = hp.tile([P, MF, C], b16)

            # Stage 1: h = silu(Wg^T x) * (Wu^T x), laid out [ff-part, C]
            with tc.tile_pool(name="ps1", bufs=NT, space="PSUM") as ps1:
                for mf in range(MF):
                    wt = w1p.tile([P, 2, KD, P], b16)
                    nc.sync.dma_start(wt[:], w1[mf])
                    psg = [ps1.tile([P, 512], f32, tag="psg", name=f"psg_{mf}_{t}")
                           for t in range(NT)]
                    psu = [ps1.tile([P, 512], f32, tag="psu", name=f"psu_{mf}_{t}")
                           for t in range(NT)]
                    for kd in range(KD):
                        for gu in range(2):
                            ps = psg if gu == 0 else psu
                            for t in range(NT):
                                nc.tensor.matmul(
                                    ps[t][:, :tiles[t]],
                                    wt[:, gu, kd],
                                    x_sb[:, kd, offs[t]:offs[t] + tiles[t]],
                                    start=(kd == 0),
                                    stop=(kd == KD - 1),
                                )
                    for t in range(NT):
                        sil = silp.tile([P, TSMAX], f32, tag="sil",
                                        name=f"sil_{mf}_{t}")
                        nc.scalar.activation(
                            sil[:, :tiles[t]], psg[t][:, :tiles[t]],
                            mybir.ActivationFunctionType.Silu,
                        )
                        nc.vector.tensor_mul(
                            h_sb[:, mf, offs[t]:offs[t] + tiles[t]],
                            sil[:, :tiles[t]], psu[t][:, :tiles[t]],
                        )

            # Stage 2: y = Wd^T h, laid out [d-part, C]
            with tc.tile_pool(name="ps2", bufs=NT, space="PSUM") as ps2:
                for md in range(KD):
                    wdt = wdpool.tile([P, MF, P], b16)
                    nc.sync.dma_start(wdt[:], wdp[md])
                    y_sb = yp.tile([P, C], b16)
                    psy = [ps2.tile([P, 512], f32, tag="psy", name=f"psy_{md}_{t}")
                           for t in range(NT)]
                    for kf in range(MF):
                        for t in range(NT):
                            nc.tensor.matmul(
                                psy[t][:, :tiles[t]],
                                wdt[:, kf],
                                h_sb[:, kf, offs[t]:offs[t] + tiles[t]],
                                start=(kf == 0),
                                stop=(kf == MF - 1),
                            )
                    for t in range(NT):
                        nc.vector.tensor_copy(
                            y_sb[:, offs[t]:offs[t] + tiles[t]],
                            psy[t][:, :tiles[t]],
                        )
                    nc.sync.dma_start(yT[md], y_sb[:])

    nc.compile()
    return nc


def _pack_w1(wg: np.ndarray, wu: np.ndarray) -> np.ndarray:
    """[D, F] x2 -> [MF, P, 2, KD, P] bf16, matching the kernel's layout."""
    # w1[mf, p, gu, kd, c] = w_gu[kd*128 + p, mf*128 + c]
    stack = np.stack([wg, wu], axis=0)            # [2, D, F]
    r = stack.reshape(2, KD, P, MF, P)            # [gu, kd, p, mf, c]
    return np.ascontiguousarray(r.transpose(3, 2, 0, 1, 4)).astype(BF16)


def _pack_wd(wd: np.ndarray) -> np.ndarray:
    """[F, D] -> [KD, P, MF, P] bf16. wdp[md, p, kf, c] = wd[kf*128+p, md*128+c]"""
    r = wd.reshape(MF, P, KD, P)                  # [kf, p, md, c]
    return np.ascontiguousarray(r.transpose(2, 1, 0, 3)).astype(BF16)


def _run_device(in_maps, tiles):
    from concourse.bass_utils import run_bass_kernel_spmd

    if tiles not in _program_cache:
        _program_cache[tiles] = _build_program(tiles)
    nc = _program_cache[tiles]
    res = run_bass_kernel_spmd(nc, in_maps, core_ids=list(range(N_CORES)))
    return [r["yT"] for r in res.results]


def kernel(hidden_states, routing_mask, wg0, wu0, wd0, wg1, wu1, wd1,
           _run=None):
    hidden_states = np.asarray(hidden_states, dtype=np.float32)
    routing_mask = np.asarray(routing_mask)
    B, S, D = hidden_states.shape
    NTOK = B * S
    x = hidden_states.reshape(NTOK, D)
    mask = routing_mask.reshape(NTOK)

    idx = [np.nonzero(mask == e)[0] for e in (0, 1)]
    maxpc = max(
        (len(idx[0]) + CPE - 1) // CPE,
        (len(idx[1]) + CPE - 1) // CPE,
        1,
    )
    tiles = _token_tiles(maxpc)
    C = sum(tiles)

    w1_packed = [_pack_w1(np.asarray(wg0), np.asarray(wu0)),
                 _pack_w1(np.asarray(wg1), np.asarray(wu1))]
    wd_packed = [_pack_wd(np.asarray(wd0)), _pack_wd(np.asarray(wd1))]

    in_maps = []
    chunks = []  # (expert, token_indices) per core
    for core in range(N_CORES):
        e = core // CPE
        slot = core % CPE
        ids = idx[e]
        # split ids into CPE nearly-equal chunks
        bounds = [(len(ids) * i) // CPE for i in range(CPE + 1)]
        ids_c = ids[bounds[slot]:bounds[slot + 1]]
        chunks.append((e, ids_c))

        xc = np.zeros((C, D), dtype=np.float32)
        xc[: len(ids_c)] = x[ids_c]
        # xT[p, kd, c] = xc[c, kd*128 + p]
        xT = np.ascontiguousarray(
            xc.reshape(C, KD, P).transpose(2, 1, 0)
        ).astype(BF16)
        in_maps.append({
            "xT": xT,
            "w1": w1_packed[e],
            "wd": wd_packed[e],
        })

    run = _run if _run is not None else _run_device
    outs = run(in_maps, tiles)

    y_full = np.zeros((NTOK, D), dtype=np.float32)
    for core in range(N_CORES):
        _, ids_c = chunks[core]
        if len(ids_c) == 0:
            continue
        yT = np.asarray(outs[core]).astype(np.float32).reshape(D, C)
        y_full[ids_c] = yT[:, : len(ids_c)].T
    return y_full.reshape(B, S, D)


# revision 9
# speedup vs baseline: 1.0331x; 1.0061x over previous
"""MoE (2-expert SwiGLU) Trainium2 kernel, 8-core SPMD.

Strategy: since the MLPs have no biases and silu(0) = 0, MLP(0) = 0, so each
token only needs the expert it is routed to.  The host gathers tokens by
expert (MoE dispatch), cores 0-3 process expert-0 tokens and cores 4-7
expert-1 tokens (~1/8 of total tokens per core), each core running a dense
SwiGLU MLP with its expert's weights.  The host scatters per-core outputs
back into the full [B, S, D] output.  This halves FLOPs vs. the reference's
dense-masked formulation and needs no collectives.

Device dataflow (per core, transposed so no on-chip transposes are needed):
  yT = Wd^T @ (silu(Wg^T @ xT) * (Wu^T @ xT))
Weights are the stationary matmul operand, token-columns the moving operand.
All matmuls are bf16 with fp32 PSUM accumulation.  The FF intermediate `h`
for all of a core's tokens stays resident in SBUF, so each weight byte is
DMA'd exactly once per core.
"""

import sys

for _p in ("/opt/trn_rl_repo", "/root/.axon_site/_ro/trn_rl_repo"):
    if _p not in sys.path:
        sys.path.append(_p)

import numpy as np
import ml_dtypes

BF16 = ml_dtypes.bfloat16

D_MODEL = 1024
D_FF = 4096
P = 128
KD = D_MODEL // P  # 8   k-tiles over d_model
MF = D_FF // P     # 32  tiles over d_ff
N_CORES = 8
CPE = 4            # cores per expert
NT = 3             # token tiles per core

_program_cache: dict[tuple, object] = {}


def _token_tiles(maxpc: int) -> tuple:
    """Split the per-core token capacity into NT near-equal tiles (each a
    multiple of 8 except possibly the last, each <= 512)."""
    C = 8 * ((maxpc + 7) // 8)
    C = max(C, 24)
    t = 8 * ((C + NT * 8 - 1) // (NT * 8))    # per-tile, rounded up to 8
    tiles = []
    left = C
    for _ in range(NT):
        s = min(t, left)
        tiles.append(s)
        left -= s
    assert left == 0 and all(0 < s <= 512 for s in tiles), (maxpc, tiles)
    return tuple(tiles)


def _build_program(tiles: tuple):
    """Bass program for one core: x [D,C] -> y [D,C], C = sum(tiles) tokens."""
    import concourse.tile as tile
    from concourse import mybir, bacc

    C = sum(tiles)
    offs = [sum(tiles[:i]) for i in range(len(tiles))]
    TSMAX = max(tiles)
    f32 = mybir.dt.float32
    b16 = mybir.dt.bfloat16

    nc = bacc.Bacc()
    xT = nc.declare_dram_parameter("xT", [P, KD, C], b16, isOutput=False)
    # w1[mf, p, gu, kd, c] = (wg if gu==0 else wu)[kd*128 + p, mf*128 + c]
    w1 = nc.declare_dram_parameter("w1", [MF, P, 2, KD, P], b16, isOutput=False)
    # wdp[md, p, kf, c] = wd[kf*128 + p, md*128 + c]
    wdp = nc.declare_dram_parameter("wd", [KD, P, MF, P], b16, isOutput=False)
    yT = nc.declare_dram_parameter("yT", [KD, P, C], b16, isOutput=True)

    with tile.TileContext(nc) as tc:
        with (
            tc.tile_pool(name="xp", bufs=1) as xp,
            tc.tile_pool(name="hp", bufs=1) as hp,
            tc.tile_pool(name="w1p", bufs=3) as w1p,
            tc.tile_pool(name="wdpool", bufs=2) as wdpool,
            tc.tile_pool(name="silp", bufs=4) as silp,
            tc.tile_pool(name="yp", bufs=2) as yp,
        ):
            x_sb = xp.tile([P, KD, C], b16)
            h_sb = hp.tile([P, MF, C], b16)
            # The sync HWDGE ring drains in issue order, so stage the startup
            # transfers in the order the PE consumes them: first weight tile,
            # then x k-slices in growing chunks.
            wt0 = w1p.tile([P, 2, KD, P], b16, tag="wt", name="wt_0")
            nc.sync.dma_start(wt0[:], w1[0])
            nc.sync.dma_start(x_sb[:, 0], xT[:, 0])
            nc.sync.dma_start(x_sb[:, 1], xT[:, 1])
            nc.sync.dma_start(x_sb[:, 2:4], xT[:, 2:4])
            nc.sync.dma_start(x_sb[:, 4:], xT[:, 4:])

            # Stage 1: h = silu(Wg^T x) * (Wu^T x), laid out [ff-part, C]
            with tc.tile_pool(name="ps1", bufs=NT, space="PSUM") as ps1:
                for mf in range(MF):
                    if mf == 0:
                        wt = wt0
                    else:
                        wt = w1p.tile([P, 2, KD, P], b16, tag="wt",
                                      name=f"wt_{mf}")
                        nc.sync.dma_start(wt[:], w1[mf])
                    psg = [ps1.tile([P, 512], f32, tag="psg", name=f"psg_{mf}_{t}")
                           for t in range(NT)]
                    psu = [ps1.tile([P, 512], f32, tag="psu", name=f"psu_{mf}_{t}")
                           for t in range(NT)]
                    for kd in range(KD):
                        for gu in range(2):
                            ps = psg if gu == 0 else psu
                            for t in range(NT):
                                nc.tensor.matmul(
                                    ps[t][:, :tiles[t]],
                                    wt[:, gu, kd],
                                    x_sb[:, kd, offs[t]:offs[t] + tiles[t]],
                                    start=(kd == 0),
                                    stop=(kd == KD - 1),
                                )
                    for t in range(NT):
                        sil = silp.tile([P, TSMAX], f32, tag="sil",
                                        name=f"sil_{mf}_{t}")
                        nc.scalar.activation(
                            sil[:, :tiles[t]], psg[t][:, :tiles[t]],
                            mybir.ActivationFunctionType.Silu,
                        )
                        nc.vector.tensor_mul(
                            h_sb[:, mf, offs[t]:offs[t] + tiles[t]],
                            sil[:, :tiles[t]], psu[t][:, :tiles[t]],
                        )

            # Stage 2: y = Wd^T h, laid out [d-part, C]
            with tc.tile_pool(name="ps2", bufs=NT, space="PSUM") as ps2:
                for md in range(KD):
                    wdt = wdpool.tile([P, MF, P], b16)
                    nc.sync.dma_start(wdt[:], wdp[md])
                    y_sb = yp.tile([P, C], b16)
                    psy = [ps2.tile([P, 512], f32, tag="psy", name=f"psy_{md}_{t}")
                           for t in range(NT)]
                    for kf in range(MF):
                        for t in range(NT):
                            nc.tensor.matmul(
                                psy[t][:, :tiles[t]],
                                wdt[:, kf],
                                h_sb[:, kf, offs[t]:offs[t] + tiles[t]],
                                start=(kf == 0),
                                stop=(kf == MF - 1),
                            )
                    for t in range(NT):
                        nc.vector.tensor_copy(
                            y_sb[:, offs[t]:offs[t] + tiles[t]],
                            psy[t][:, :tiles[t]],
                        )
                    nc.sync.dma_start(yT[md], y_sb[:])

    nc.compile()
    return nc


def _pack_w1(wg: np.ndarray, wu: np.ndarray) -> np.ndarray:
    """[D, F] x2 -> [MF, P, 2, KD, P] bf16, matching the kernel's layout."""
    # w1[mf, p, gu, kd, c] = w_gu[kd*128 + p, mf*128 + c]
    stack = np.stack([wg, wu], axis=0)            # [2, D, F]
    r = stack.reshape(2, KD, P, MF, P)            # [gu, kd, p, mf, c]
    return np.ascontiguousarray(r.transpose(3, 2, 0, 1, 4)).astype(BF16)


def _pack_wd(wd: np.ndarray) -> np.ndarray:
    """[F, D] -> [KD, P, MF, P] bf16. wdp[md, p, kf, c] = wd[kf*128+p, md*128+c]"""
    r = wd.reshape(MF, P, KD, P)                  # [kf, p, md, c]
    return np.ascontiguousarray(r.transpose(2, 1, 0, 3)).astype(BF16)


def _run_device(in_maps, tiles):
    from concourse.bass_utils import run_bass_kernel_spmd

    if tiles not in _program_cache:
        _program_cache[tiles] = _build_program(tiles)
    nc = _program_cache[tiles]
    res = run_bass_kernel_spmd(nc, in_maps, core_ids=list(range(N_CORES)))
    return [r["yT"] for r in res.results]


def kernel(hidden_states, routing_mask, wg0, wu0, wd0, wg1, wu1, wd1,
           _run=None):
    hidden_states = np.asarray(hidden_states, dtype=np.float32)
    routing_mask = np.asarray(routing_mask)
    B, S, D = hidden_states.shape
    NTOK = B * S
    x = hidden_states.reshape(NTOK, D)
    mask = routing_mask.reshape(NTOK)

    idx = [np.nonzero(mask == e)[0] for e in (0, 1)]
    maxpc = max(
        (len(idx[0]) + CPE - 1) // CPE,
        (len(idx[1]) + CPE - 1) // CPE,
        1,
    )
    tiles = _token_tiles(maxpc)
    C = sum(tiles)

    w1_packed = [_pack_w1(np.asarray(wg0), np.asarray(wu0)),
                 _pack_w1(np.asarray(wg1), np.asarray(wu1))]
    wd_packed = [_pack_wd(np.asarray(wd0)), _pack_wd(np.asarray(wd1))]

    in_maps = []
    chunks = []  # (expert, token_indices) per core
    for core in range(N_CORES):
        e = core // CPE
        slot = core % CPE
        ids = idx[e]
        # split ids into CPE nearly-equal chunks
        bounds = [(len(ids) * i) // CPE for i in range(CPE + 1)]
        ids_c = ids[bounds[slot]:bounds[slot + 1]]
        chunks.append((e, ids_c))

        xc = np.zeros((C, D), dtype=np.float32)
        xc[: len(ids_c)] = x[ids_c]
        # xT[p, kd, c] = xc[c, kd*128 + p]
        xT = np.ascontiguousarray(
            xc.reshape(C, KD, P).transpose(2, 1, 0)
        ).astype(BF16)
        in_maps.append({
            "xT": xT,
            "w1": w1_packed[e],
            "wd": wd_packed[e],
        })

    run = _run if _run is not None else _run_device
    outs = run(in_maps, tiles)

    y_full = np.zeros((NTOK, D), dtype=np.float32)
    for core in range(N_CORES):
        _, ids_c = chunks[core]
        if len(ids_c) == 0:
            continue
        yT = np.asarray(outs[core]).astype(np.float32).reshape(D, C)
        y_full[ids_c] = yT[:, : len(ids_c)].T
    return y_full.reshape(B, S, D)


# revision 10
# speedup vs baseline: 1.0493x; 1.0157x over previous
"""MoE (2-expert SwiGLU) Trainium2 kernel, 8-core SPMD.

Strategy: since the MLPs have no biases and silu(0) = 0, MLP(0) = 0, so each
token only needs the expert it is routed to.  The host gathers tokens by
expert (MoE dispatch), cores 0-3 process expert-0 tokens and cores 4-7
expert-1 tokens (~1/8 of total tokens per core), each core running a dense
SwiGLU MLP with its expert's weights.  The host scatters per-core outputs
back into the full [B, S, D] output.  This halves FLOPs vs. the reference's
dense-masked formulation and needs no collectives.

Device dataflow (per core, transposed so no on-chip transposes are needed):
  yT = Wd^T @ (silu(Wg^T @ xT) * (Wu^T @ xT))
Weights are the stationary matmul operand, token-columns the moving operand.
All matmuls are bf16 with fp32 PSUM accumulation.  The FF intermediate `h`
for all of a core's tokens stays resident in SBUF, so each weight byte is
DMA'd exactly once per core.
"""

import sys

for _p in ("/opt/trn_rl_repo", "/root/.axon_site/_ro/trn_rl_repo"):
    if _p not in sys.path:
        sys.path.append(_p)

import numpy as np
import ml_dtypes

BF16 = ml_dtypes.bfloat16

D_MODEL = 1024
D_FF = 4096
P = 128
KD = D_MODEL // P  # 8   k-tiles over d_model
MF = D_FF // P     # 32  tiles over d_ff
N_CORES = 8
CPE = 4            # cores per expert
NT = 3             # token tiles per core

_program_cache: dict[tuple, object] = {}


def _token_tiles(maxpc: int) -> tuple:
    """Split the per-core token capacity into NT near-equal tiles (each a
    multiple of 8 except possibly the last, each <= 512)."""
    C = 8 * ((maxpc + 7) // 8)
    C = max(C, 24)
    t = 8 * ((C + NT * 8 - 1) // (NT * 8))    # per-tile, rounded up to 8
    tiles = []
    left = C
    for _ in range(NT):
        s = min(t, left)
        tiles.append(s)
        left -= s
    assert left == 0 and all(0 < s <= 512 for s in tiles), (maxpc, tiles)
    return tuple(tiles)


def _build_program(tiles: tuple):
    """Bass program for one core: x [D,C] -> y [D,C], C = sum(tiles) tokens."""
    import concourse.tile as tile
    from concourse import mybir, bacc

    C = sum(tiles)
    offs = [sum(tiles[:i]) for i in range(len(tiles))]
    TSMAX = max(tiles)
    f32 = mybir.dt.float32
    b16 = mybir.dt.bfloat16

    nc = bacc.Bacc()
    xT = nc.declare_dram_parameter("xT", [P, KD, C], b16, isOutput=False)
    # w1[mf, p, gu, kd, c] = (wg if gu==0 else wu)[kd*128 + p, mf*128 + c]
    w1 = nc.declare_dram_parameter("w1", [MF, P, 2, KD, P], b16, isOutput=False)
    # wdp[md, p, kf, c] = wd[kf*128 + p, md*128 + c]
    wdp = nc.declare_dram_parameter("wd", [KD, P, MF, P], b16, isOutput=False)
    yT = nc.declare_dram_parameter("yT", [KD, P, C], b16, isOutput=True)

    with tile.TileContext(nc) as tc:
        with (
            tc.tile_pool(name="xp", bufs=1) as xp,
            tc.tile_pool(name="hp", bufs=1) as hp,
            tc.tile_pool(name="w1p", bufs=3) as w1p,
            tc.tile_pool(name="wdpool", bufs=2) as wdpool,
            tc.tile_pool(name="silp", bufs=4) as silp,
            tc.tile_pool(name="yp", bufs=2) as yp,
        ):
            x_sb = xp.tile([P, KD, C], b16)
            h_sb = hp.tile([P, MF, C], b16)
            # The sync HWDGE ring drains in issue order, so stage the startup
            # transfers in the order the PE consumes them: first weight tile,
            # then x k-slices in growing chunks.
            wt0 = w1p.tile([P, 2, KD, P], b16, tag="wt", name="wt_0")
            nc.sync.dma_start(wt0[:], w1[0])
            nc.sync.dma_start(x_sb[:, 0], xT[:, 0])
            nc.sync.dma_start(x_sb[:, 1], xT[:, 1])
            nc.sync.dma_start(x_sb[:, 2:4], xT[:, 2:4])
            nc.sync.dma_start(x_sb[:, 4:], xT[:, 4:])

            # Stage 1: h = silu(Wg^T x) * (Wu^T x), laid out [ff-part, C]
            with tc.tile_pool(name="ps1", bufs=NT, space="PSUM") as ps1:
                for mf in range(MF):
                    if mf == 0:
                        wt = wt0
                    else:
                        wt = w1p.tile([P, 2, KD, P], b16, tag="wt",
                                      name=f"wt_{mf}")
                        nc.sync.dma_start(wt[:], w1[mf])
                    psg = [ps1.tile([P, 512], f32, tag="psg", name=f"psg_{mf}_{t}")
                           for t in range(NT)]
                    psu = [ps1.tile([P, 512], f32, tag="psu", name=f"psu_{mf}_{t}")
                           for t in range(NT)]
                    for kd in range(KD):
                        for gu in range(2):
                            ps = psg if gu == 0 else psu
                            for t in range(NT):
                                nc.tensor.matmul(
                                    ps[t][:, :tiles[t]],
                                    wt[:, gu, kd],
                                    x_sb[:, kd, offs[t]:offs[t] + tiles[t]],
                                    start=(kd == 0),
                                    stop=(kd == KD - 1),
                                )
                    for t in range(NT):
                        sil = silp.tile([P, TSMAX], f32, tag="sil",
                                        name=f"sil_{mf}_{t}")
                        nc.scalar.activation(
                            sil[:, :tiles[t]], psg[t][:, :tiles[t]],
                            mybir.ActivationFunctionType.Silu,
                        )
                        nc.vector.tensor_mul(
                            h_sb[:, mf, offs[t]:offs[t] + tiles[t]],
                            sil[:, :tiles[t]], psu[t][:, :tiles[t]],
                        )

            # Stage 2: y = Wd^T h, laid out [d-part, C]
            with tc.tile_pool(name="ps2", bufs=2 * NT, space="PSUM") as ps2:
                for md in range(KD):
                    wdt = wdpool.tile([P, MF, P], b16)
                    nc.sync.dma_start(wdt[:], wdp[md])
                    y_sb = yp.tile([P, C], b16)
                    psy = [ps2.tile([P, 512], f32, tag="psy", name=f"psy_{md}_{t}")
                           for t in range(NT)]
                    for kf in range(MF):
                        for t in range(NT):
                            nc.tensor.matmul(
                                psy[t][:, :tiles[t]],
                                wdt[:, kf],
                                h_sb[:, kf, offs[t]:offs[t] + tiles[t]],
                                start=(kf == 0),
                                stop=(kf == MF - 1),
                            )
                    for t in range(NT):
                        nc.vector.tensor_copy(
                            y_sb[:, offs[t]:offs[t] + tiles[t]],
                            psy[t][:, :tiles[t]],
                        )
                        # per-tile store so the final DMA after the last
                        # matmul is small
                        nc.sync.dma_start(
                            yT[md, :, offs[t]:offs[t] + tiles[t]],
                            y_sb[:, offs[t]:offs[t] + tiles[t]],
                        )

    nc.compile()
    return nc


def _pack_w1(wg: np.ndarray, wu: np.ndarray) -> np.ndarray:
    """[D, F] x2 -> [MF, P, 2, KD, P] bf16, matching the kernel's layout."""
    # w1[mf, p, gu, kd, c] = w_gu[kd*128 + p, mf*128 + c]
    stack = np.stack([wg, wu], axis=0)            # [2, D, F]
    r = stack.reshape(2, KD, P, MF, P)            # [gu, kd, p, mf, c]
    return np.ascontiguousarray(r.transpose(3, 2, 0, 1, 4)).astype(BF16)


def _pack_wd(wd: np.ndarray) -> np.ndarray:
    """[F, D] -> [KD, P, MF, P] bf16. wdp[md, p, kf, c] = wd[kf*128+p, md*128+c]"""
    r = wd.reshape(MF, P, KD, P)                  # [kf, p, md, c]
    return np.ascontiguousarray(r.transpose(2, 1, 0, 3)).astype(BF16)


def _run_device(in_maps, tiles):
    from concourse.bass_utils import run_bass_kernel_spmd

    if tiles not in _program_cache:
        _program_cache[tiles] = _build_program(tiles)
    nc = _program_cache[tiles]
    res = run_bass_kernel_spmd(nc, in_maps, core_ids=list(range(N_CORES)))
    return [r["yT"] for r in res.results]


def kernel(hidden_states, routing_mask, wg0, wu0, wd0, wg1, wu1, wd1,
           _run=None):
    hidden_states = np.asarray(hidden_states, dtype=np.float32)
    routing_mask = np.asarray(routing_mask)
    B, S, D = hidden_states.shape
    NTOK = B * S
    x = hidden_states.reshape(NTOK, D)
    mask = routing_mask.reshape(NTOK)

    idx = [np.nonzero(mask == e)[0] for e in (0, 1)]
    maxpc = max(
        (len(idx[0]) + CPE - 1) // CPE,
        (len(idx[1]) + CPE - 1) // CPE,
        1,
    )
    tiles = _token_tiles(maxpc)
    C = sum(tiles)

    w1_packed = [_pack_w1(np.asarray(wg0), np.asarray(wu0)),
                 _pack_w1(np.asarray(wg1), np.asarray(wu1))]
    wd_packed = [_pack_wd(np.asarray(wd0)), _pack_wd(np.asarray(wd1))]

    in_maps = []
    chunks = []  # (expert, token_indices) per core
    for core in range(N_CORES):
        e = core // CPE
        slot = core % CPE
        ids = idx[e]
        # split ids into CPE nearly-equal chunks
        bounds = [(len(ids) * i) // CPE for i in range(CPE + 1)]
        ids_c = ids[bounds[slot]:bounds[slot + 1]]
        chunks.append((e, ids_c))

        xc = np.zeros((C, D), dtype=np.float32)
        xc[: len(ids_c)] = x[ids_c]
        # xT[p, kd, c] = xc[c, kd*128 + p]
        xT = np.ascontiguousarray(
            xc.reshape(C, KD, P).transpose(2, 1, 0)
        ).astype(BF16)
        in_maps.append({
            "xT": xT,
            "w1": w1_packed[e],
            "wd": wd_packed[e],
        })

    run = _run if _run is not None else _run_device
    outs = run(in_maps, tiles)

    y_full = np.zeros((NTOK, D), dtype=np.float32)
    for core in range(N_CORES):
        _, ids_c = chunks[core]
        if len(ids_c) == 0:
            continue
        yT = np.asarray(outs[core]).astype(np.float32).reshape(D, C)
        y_full[ids_c] = yT[:, : len(ids_c)].T
    return y_full.reshape(B, S, D)
